# revision 1
# baseline (speedup 1.0000x reference)
"""Trainium2 Bass kernel for relational graph convolution:

    y = sum_r (A[r] @ x) @ W[r].T        A: [8, 4096, 4096] f32
                                         x: [4096, 64] f32, W: [8, 64, 64] f32

Strategy
--------
By associativity, y = sum_r A[r] @ v_r with v_r = x @ W[r].T, turning the
problem into one [4096, 4096] @ [4096, 64] matmul per relation. Relations are
sharded across the 8 NeuronCores (expert-style parallelism); each core returns
its partial y_r.T and the host sums and transposes.

The TensorE contracts over the partition dimension of both operands, so the
contraction index m (A's column index) must land on SBUF partitions. The host
therefore ships A[r].T (row-major) so device DMAs are plain contiguous slabs.

Per core:
  phase 1: v = x @ W_r.T via 32 exact-fp32 matmuls (lhsT = x.T column chunks,
           rhs = W_r.T), rounded into a float32r SBUF tile.
  phase 2: for each of 32 contraction chunks (128 rows of A_r.T): one 2 MB DMA,
           then 8 float32r matmuls (lhsT = v chunk [128, 64], rhs = A_r.T slab
           [128, 512]) accumulating y_r.T [64, 4096] across all 8 PSUM banks.
  phase 3: per-bank PSUM -> SBUF copies chase the final matmuls, then one DMA
           of y_r.T out.

float32r (4-byte, reduced-mantissa matmul mode) streams at 1 cycle/row vs 4 for
float32, making the kernel DMA-bound (~64 MB/core of A traffic, measured
~427 GB/s/core with all 8 cores streaming) instead of PE-bound; measured
end-to-end accuracy is ~1e-4 relative.

MODE="bf16" is an optional variant that ships A as bf16 (halving DMA traffic);
~3x the error, kept behind a flag.
"""

import numpy as np

import concourse.tile as tile
from concourse import bacc, mybir
from concourse.bass_utils import run_bass_kernel_spmd

R, N, IN_F, OUT_F = 8, 4096, 64, 64
P = 128            # partition dim / contraction chunk
MC = N // P        # 32 contraction chunks
BANK = 512         # fp32 elems per PSUM bank
NB = N // BANK     # 8 output column blocks

F32 = mybir.dt.float32

MODE = "f32r"      # "f32r" (default) or "bf16"

_NC_CACHE = {}


def _build_nc(repeat=1, mode=None, jc=None, alt=True, at_bufs=None):
    """repeat>1 re-runs phase 2 (the steady-state A-streaming loop) that many
    times inside one NEFF — used only by the benchmark harness to amortize
    per-execute dispatch overhead; the graded kernel uses repeat=1.

    jc = 128-row chunks of A per DMA transfer; alt = alternate the two HWDGE
    rings (SP / ACT) between consecutive A-slab DMAs to hide the per-DMA
    completion gap."""
    mode = mode or MODE
    a_dt = mybir.dt.float32r if mode == "f32r" else mybir.dt.bfloat16
    if jc is None:
        jc = 1 if mode == "f32r" else 2
    if at_bufs is None:
        at_bufs = {1: 4, 2: 3, 4: 2}[jc] if mode == "f32r" else 4

    nc = bacc.Bacc("TRN2", target_bir_lowering=False, debug=False, num_devices=R)

    at = nc.dram_tensor("at", [N, N], a_dt, kind="ExternalInput").ap()
    xt = nc.dram_tensor("xt", [IN_F, N], F32, kind="ExternalInput").ap()
    wt = nc.dram_tensor("wt", [IN_F, OUT_F], F32, kind="ExternalInput").ap()
    ytp = nc.dram_tensor("ytp", [OUT_F, N], F32, kind="ExternalOutput").ap()

    with tile.TileContext(nc) as tc:
        with (
            tc.tile_pool(name="const", bufs=1) as const_pool,
            tc.tile_pool(name="atp", bufs=at_bufs) as at_pool,
            tc.tile_pool(name="vp", bufs=2) as v_pool,
            tc.tile_pool(name="outp", bufs=2) as out_pool,
        ):
            xt_sb = const_pool.tile([IN_F, N], F32)
            nc.sync.dma_start(xt_sb[:], xt[:])
            wt_sb = const_pool.tile([IN_F, OUT_F], F32)
            nc.sync.dma_start(wt_sb[:], wt[:])

            at_r3 = at.rearrange("(c j p) n -> c p j n", p=P, j=jc)

            # phase 1: v[m, o] = sum_i x[m, i] W[o, i], exact fp32, then
            # rounded to the matmul dtype by the DVE copy.
            v_sb = v_pool.tile([P, MC, OUT_F], a_dt, tag="v_sb")
            with tc.tile_pool(name="psv", bufs=2, space="PSUM") as psv_pool:
                for mc in range(MC):
                    ps_v = psv_pool.tile([P, OUT_F], F32)
                    nc.tensor.matmul(
                        ps_v[:],
                        xt_sb[:, mc * P : (mc + 1) * P],
                        wt_sb[:],
                        start=True,
                        stop=True,
                    )
                    nc.vector.tensor_copy(v_sb[:, mc, :], ps_v[:])

            # phase 2: y_r.T[o, n] += sum_m v[m, o] * A_r.T[m, n]
            with tc.tile_pool(name="psy", bufs=1, space="PSUM") as psy_pool:
                for _rep in range(repeat):
                    out_sb = out_pool.tile([OUT_F, N], F32, tag="out_sb")
                    ps_y = psy_pool.tile([OUT_F, N], F32, tag="ps_y")
                    for c in range(MC // jc):
                        at_t = at_pool.tile([P, jc, N], a_dt)
                        eng = nc.scalar if (alt and c % 2) else nc.sync
                        eng.dma_start(at_t[:], at_r3[c])
                        for j in range(jc):
                            mc = c * jc + j
                            for b in range(NB):
                                nc.tensor.matmul(
                                    ps_y[:, b * BANK : (b + 1) * BANK],
                                    v_sb[:, mc, :],
                                    at_t[:, j, b * BANK : (b + 1) * BANK],
                                    start=(mc == 0),
                                    stop=(mc == MC - 1),
                                )
                                # phase 3: per-bank copy + store chase the
                                # final matmuls
                                if mc == MC - 1:
                                    nc.vector.tensor_copy(
                                        out_sb[:, b * BANK : (b + 1) * BANK],
                                        ps_y[:, b * BANK : (b + 1) * BANK],
                                    )
                                    nc.sync.dma_start(
                                        ytp[:, b * BANK : (b + 1) * BANK],
                                        out_sb[:, b * BANK : (b + 1) * BANK],
                                    )

    nc.compile()
    return nc


def run_with_results(inputs, repeat=1, mode=None):
    """Run the kernel; returns (full_output [4096, 64] f32, BassKernelResults)."""
    mode = mode or MODE
    adjacency = np.asarray(inputs["adjacency"], dtype=np.float32)
    x = np.asarray(inputs["x"], dtype=np.float32)
    weight = np.asarray(inputs["weight"], dtype=np.float32)
    assert adjacency.shape == (R, N, N)
    assert x.shape == (N, IN_F)
    assert weight.shape == (R, OUT_F, IN_F)

    in_maps = make_in_maps(adjacency, x, weight, mode)

    key = (repeat, mode)
    if key not in _NC_CACHE:
        _NC_CACHE[key] = _build_nc(repeat, mode)
    nc = _NC_CACHE[key]

    res = run_bass_kernel_spmd(nc, in_maps, core_ids=list(range(R)))
    return assemble_output(res.results), res


def make_in_maps(adjacency, x, weight, mode=None):
    mode = mode or MODE
    # Host-side layout prep: contraction dim must land on SBUF partitions.
    at_np = np.ascontiguousarray(adjacency.transpose(0, 2, 1))  # [R, m, n]
    if mode == "bf16":
        import ml_dtypes

        at_np = at_np.astype(ml_dtypes.bfloat16)
    xt_np = np.ascontiguousarray(x.T)                           # [IN_F, N]
    wt_np = np.ascontiguousarray(weight.transpose(0, 2, 1))     # [R, IN_F, OUT_F]
    return [{"at": at_np[r], "xt": xt_np, "wt": wt_np[r]} for r in range(R)]


def assemble_output(results):
    yt = np.zeros((OUT_F, N), dtype=np.float32)
    for r in range(R):
        yt += results[r]["ytp"]
    return np.ascontiguousarray(yt.T)


def kernel(**inputs) -> np.ndarray:
    y, _ = run_with_results(inputs)
    return y



# revision 18
# speedup vs baseline: 2.3762x; 2.3762x over previous
"""Trainium2 Bass kernel for relational graph convolution:

    y = sum_r (A[r] @ x) @ W[r].T        A: [8, 4096, 4096] f32
                                         x: [4096, 64] f32, W: [8, 64, 64] f32

Strategy
--------
By associativity, y = sum_r A[r] @ v_r with v_r = x @ W[r].T, turning the
problem into one [4096, 4096] @ [4096, 64] matmul per relation. Relations are
sharded across the 8 NeuronCores (expert-style parallelism); each core returns
its partial y_r.T and the host sums and transposes.

The TensorE contracts over the partition dimension of both operands, so the
contraction index m (A's column index) must land on SBUF partitions. The host
therefore ships A[r].T (row-major) so device DMAs are plain contiguous slabs.

MODE="fp8" (default): A is shipped as fp8 e3m4 (1 byte/elem, 16 MB/core --
quarter of the f32 DMA traffic; host-simulated end-to-end rel error ~4.5e-3,
comfortably under the 2e-2 gate because the stationary operand v stays bf16 --
bass allows mixed-dtype matmuls for all non-fp32 dtypes). The PE side uses
2x column tiling: two concurrent M=64 matmuls on PE column groups 0-63 /
64-127 (tile_position (0,0) and (0,64)) stream two different 512-col slabs of
A_r.T simultaneously, halving PE streaming time to ~28 us so the kernel rides
the ~427 GB/s/core DMA roofline (~39 us for 16.5 MB).

Per core (fp8 mode):
  phase 1: v = x @ W_r.T via 32 bf16 matmuls (lhsT = x.T column chunks,
           rhs = W_r.T), rounded to bf16 in SBUF.
  phase 2: 8 slab DMAs (2 MB each, alternating the two HWDGE rings) keep all
           16 MB of A_r.T SBUF-resident; per 128-row chunk, 8 fp8 matmuls in
           4 concurrent column-tiled pairs accumulate y_r.T as PSUM [128,2048]
           (col group 0 = output cols 0-2047, col group 1 = cols 2048-4095).
  phase 3: per-region PSUM -> SBUF copies split across DVE and ACT chase the
           final matmuls, then per-region DMAs of y_r.T out.

MODE="f32r" / MODE="bf16" are the earlier exact-ish variants (kept for
fallback / accuracy reference).
"""

import numpy as np

import concourse.tile as tile
from concourse import bacc, mybir
from concourse.bass_utils import run_bass_kernel_spmd

R, N, IN_F, OUT_F = 8, 4096, 64, 64
P = 128            # partition dim / contraction chunk
MC = N // P        # 32 contraction chunks
BANK = 512         # fp32 elems per PSUM bank
NB = N // BANK     # 8 output column blocks
NB2 = NB // 2      # 4 column blocks per PE column group (fp8 mode)
HALF = N // 2

F32 = mybir.dt.float32
BF16 = mybir.dt.bfloat16
FP8 = mybir.dt.float8e3

MODE = "fp8"       # "fp8" (default), "f32r", or "bf16"

_NC_CACHE = {}


def _build_nc_fp8(repeat=1, jc=4, at_bufs=8, probe=None):
    """fp8 e3m4 A + bf16 v, 2x column-tiled PE. jc = 128-row chunks of A per
    DMA slab (jc=4 -> 2 MB transfers). probe='dma' drops all phase-2 compute
    (pure A-stream bandwidth), probe='pe' DMAs one slab once and re-reads it
    (pure PE throughput); both are diagnostics, not graded paths."""
    nc = bacc.Bacc("TRN2", target_bir_lowering=False, debug=False, num_devices=R)

    at = nc.dram_tensor("at", [N, N], FP8, kind="ExternalInput").ap()
    xt = nc.dram_tensor("xt", [IN_F, N], BF16, kind="ExternalInput").ap()
    wt = nc.dram_tensor("wt", [IN_F, OUT_F], BF16, kind="ExternalInput").ap()
    # bf16 partial outputs: halves the store traffic; the host accumulates the
    # 8 per-relation partials in fp32 so the extra rounding is ~1e-3 rel_fro.
    ytp = nc.dram_tensor("ytp", [OUT_F, N], BF16, kind="ExternalOutput").ap()

    with tile.TileContext(nc) as tc:
        with (
            tc.tile_pool(name="const", bufs=1) as const_pool,
            tc.tile_pool(name="atp", bufs=at_bufs) as at_pool,
            tc.tile_pool(name="vp", bufs=2) as v_pool,
            tc.tile_pool(name="outp", bufs=2) as out_pool,
        ):
            xt_sb = const_pool.tile([IN_F, N], BF16)
            nc.sync.dma_start(xt_sb[:], xt[:])
            wt_sb = const_pool.tile([IN_F, OUT_F], BF16)
            nc.sync.dma_start(wt_sb[:], wt[:])

            at_r3 = at.rearrange("(c j p) n -> c p j n", p=P, j=jc)

            # phase 1: v[m, o] = sum_i x[m, i] W[o, i] in bf16.
            v_sb = v_pool.tile([P, MC, OUT_F], BF16, tag="v_sb")
            with tc.tile_pool(name="psv", bufs=2, space="PSUM") as psv_pool:
                for mc in range(MC):
                    ps_v = psv_pool.tile([P, OUT_F], F32)
                    nc.tensor.matmul(
                        ps_v[:],
                        xt_sb[:, mc * P : (mc + 1) * P],
                        wt_sb[:],
                        start=True,
                        stop=True,
                    )
                    nc.vector.tensor_copy(v_sb[:, mc, :], ps_v[:])

            # phase 2: y_r.T[o, n] += sum_m v[m, o] * A_r.T[m, n], with the
            # two PE column groups handling output columns [0, 2048) and
            # [2048, 4096) concurrently.
            at_fixed = None
            if probe == "pe":
                at_fixed = const_pool.tile([P, jc, N], FP8)
                nc.sync.dma_start(at_fixed[:], at_r3[0])
            with tc.tile_pool(name="psy", bufs=2, space="PSUM") as psy_pool:
                for _rep in range(repeat):
                    out_sb = out_pool.tile([P, HALF], BF16, tag="out_sb")
                    ps_y = psy_pool.tile([P, HALF], F32, tag="ps_y")
                    for c in range(MC // jc):
                        if probe == "pe":
                            at_t = at_fixed
                        else:
                            at_t = at_pool.tile([P, jc, N], FP8)
                            eng = nc.scalar if (c % 2) else nc.sync
                            eng.dma_start(at_t[:], at_r3[c])
                        if probe == "dma":
                            continue
                        for j in range(jc):
                            mc = c * jc + j
                            for b in range(NB2):
                                sl = slice(b * BANK, (b + 1) * BANK)
                                nc.tensor.matmul(
                                    ps_y[0:OUT_F, sl],
                                    v_sb[:, mc, :],
                                    at_t[:, j, sl],
                                    start=(mc == 0),
                                    stop=(mc == MC - 1),
                                )
                                nc.tensor.matmul(
                                    ps_y[OUT_F : 2 * OUT_F, sl],
                                    v_sb[:, mc, :],
                                    at_t[:, j, HALF + b * BANK : HALF + (b + 1) * BANK],
                                    start=(mc == 0),
                                    stop=(mc == MC - 1),
                                )
                    # phase 3: per-region copies chase the final matmuls; DVE
                    # takes col group 0, ACT col group 1, staggered two PSUM
                    # banks apart so the engines never contend on one bank.
                    if probe == "dma":
                        continue
                    # (stores ride the scalar HWDGE ring so the next rep's
                    # first A slab -- queued on the sync ring -- isn't stuck
                    # behind them)
                    for b in range(NB2):
                        sl = slice(b * BANK, (b + 1) * BANK)
                        nc.vector.tensor_copy(out_sb[0:OUT_F, sl], ps_y[0:OUT_F, sl])
                        nc.scalar.dma_start(ytp[:, sl], out_sb[0:OUT_F, sl])
                        b2 = (b + 2) % NB2
                        sl2 = slice(b2 * BANK, (b2 + 1) * BANK)
                        nc.scalar.activation(
                            out_sb[OUT_F : 2 * OUT_F, sl2],
                            ps_y[OUT_F : 2 * OUT_F, sl2],
                            mybir.ActivationFunctionType.Copy,
                        )
                        nc.scalar.dma_start(
                            ytp[:, HALF + b2 * BANK : HALF + (b2 + 1) * BANK],
                            out_sb[OUT_F : 2 * OUT_F, sl2],
                        )

    nc.compile()
    return nc


def _build_nc_fp8r(repeat=1, jc=8, at_bufs=6):
    """Row-sharded variant: each core computes a disjoint 512-row slice of y
    across ALL 8 relations (contraction K = R*N = 32768), instead of a full-y
    partial for one relation. Same 16 MB/core of A traffic, but the per-rep
    output drops from 512 KB to 128 KB (the two PE column-group partials
    [2, 64, 512], summed on the host) and the store tail is one region.

    The host ships A pre-swizzled in DMA order: stream chunk t interleaves the
    two contraction halves (t even -> chunk t/2, t odd -> chunk 128 + t/2) so
    the column-group pair (grp0, grp1) consumes consecutive stream chunks and
    compute chases the DMA stream at chunk granularity."""
    CH = 2 * MC * R // 2   # 256 stream chunks of 128 contraction rows
    PAIRS = CH // 2        # 128 concurrent matmul pairs
    CJ = CH // jc          # DMA slabs

    nc = bacc.Bacc("TRN2", target_bir_lowering=False, debug=False, num_devices=R)

    at = nc.dram_tensor("at", [CJ, jc * P * BANK], FP8, kind="ExternalInput").ap()
    xt = nc.dram_tensor("xt", [IN_F, N], BF16, kind="ExternalInput").ap()
    # wt[i, r*64+o] = W[r][o, i]: all 8 relations' W.T side by side
    wt = nc.dram_tensor("wt", [IN_F, R * OUT_F], BF16, kind="ExternalInput").ap()
    ytp = nc.dram_tensor("ytp", [2 * OUT_F, BANK], BF16, kind="ExternalOutput").ap()

    def chunk_info(t):
        grp, pair = t % 2, t // 2
        kc = pair if grp == 0 else PAIRS + pair
        r, mc = divmod(kc, MC)
        return grp, pair, r, mc

    with tile.TileContext(nc) as tc:
        with (
            tc.tile_pool(name="const", bufs=1) as const_pool,
            tc.tile_pool(name="atp", bufs=at_bufs) as at_pool,
            tc.tile_pool(name="vp", bufs=2) as v_pool,
            tc.tile_pool(name="outp", bufs=2) as out_pool,
        ):
            xt_sb = const_pool.tile([IN_F, N], BF16)
            nc.sync.dma_start(xt_sb[:], xt[:])
            wt_sb = const_pool.tile([IN_F, R * OUT_F], BF16)
            nc.sync.dma_start(wt_sb[:], wt[:])

            at_r = at.rearrange("d (p j n) -> d p j n", p=P, j=jc)

            # phase 1: v for all 8 relations; one [128, 512] matmul per row
            # chunk computes x_chunk @ [W_0.T | ... | W_7.T].
            v_sb = v_pool.tile([P, MC, R * OUT_F], BF16, tag="v_sb")
            with tc.tile_pool(name="psv", bufs=2, space="PSUM") as psv_pool:
                for mc in range(MC):
                    ps_v = psv_pool.tile([P, R * OUT_F], F32)
                    nc.tensor.matmul(
                        ps_v[:],
                        xt_sb[:, mc * P : (mc + 1) * P],
                        wt_sb[:],
                        start=True,
                        stop=True,
                    )
                    nc.vector.tensor_copy(v_sb[:, mc, :], ps_v[:])

            # phase 2: y_slice.T[o, n] = sum_{r,m} v_r[m, o] * A_r.T[m, n],
            # col group 0 accumulating stream chunks t even, group 1 t odd.
            with tc.tile_pool(name="psy", bufs=2, space="PSUM") as psy_pool:
                for _rep in range(repeat):
                    out_sb = out_pool.tile([P, BANK], BF16, tag="out_sb")
                    ps_y = psy_pool.tile([P, BANK], F32, tag="ps_y")
                    for d in range(CJ):
                        at_t = at_pool.tile([P, jc, BANK], FP8)
                        eng = nc.scalar if (d % 2) else nc.sync
                        eng.dma_start(at_t[:], at_r[d])
                        for j in range(jc):
                            t = d * jc + j
                            grp, pair, r, mc = chunk_info(t)
                            nc.tensor.matmul(
                                ps_y[grp * OUT_F : (grp + 1) * OUT_F, :],
                                v_sb[:, mc, r * OUT_F : (r + 1) * OUT_F],
                                at_t[:, j, :],
                                start=(pair == 0),
                                stop=(pair == PAIRS - 1),
                            )
                    nc.vector.tensor_copy(out_sb[0:OUT_F, :], ps_y[0:OUT_F, :])
                    nc.scalar.activation(
                        out_sb[OUT_F : 2 * OUT_F, :],
                        ps_y[OUT_F : 2 * OUT_F, :],
                        mybir.ActivationFunctionType.Copy,
                    )
                    nc.scalar.dma_start(ytp[:], out_sb[:])

    nc.compile()
    return nc


def _build_nc(repeat=1, mode=None, jc=None, alt=True, at_bufs=None):
    """repeat>1 re-runs phase 2 (the steady-state A-streaming loop) that many
    times inside one NEFF -- used only by the benchmark harness to amortize
    per-execute dispatch overhead; the graded kernel uses repeat=1."""
    mode = mode or MODE
    if mode == "fp8r":
        return _build_nc_fp8r(repeat, jc=jc or 8, at_bufs=at_bufs or 6)
    if mode.startswith("fp8"):
        probe = {"fp8": None, "fp8_dma": "dma", "fp8_pe": "pe"}[mode]
        return _build_nc_fp8(repeat, jc=jc or 4, at_bufs=at_bufs or 8, probe=probe)

    a_dt = mybir.dt.float32r if mode == "f32r" else mybir.dt.bfloat16
    if jc is None:
        jc = 1 if mode == "f32r" else 2
    if at_bufs is None:
        at_bufs = {1: 4, 2: 3, 4: 2}[jc] if mode == "f32r" else 4

    nc = bacc.Bacc("TRN2", target_bir_lowering=False, debug=False, num_devices=R)

    at = nc.dram_tensor("at", [N, N], a_dt, kind="ExternalInput").ap()
    xt = nc.dram_tensor("xt", [IN_F, N], F32, kind="ExternalInput").ap()
    wt = nc.dram_tensor("wt", [IN_F, OUT_F], F32, kind="ExternalInput").ap()
    ytp = nc.dram_tensor("ytp", [OUT_F, N], F32, kind="ExternalOutput").ap()

    with tile.TileContext(nc) as tc:
        with (
            tc.tile_pool(name="const", bufs=1) as const_pool,
            tc.tile_pool(name="atp", bufs=at_bufs) as at_pool,
            tc.tile_pool(name="vp", bufs=2) as v_pool,
            tc.tile_pool(name="outp", bufs=2) as out_pool,
        ):
            xt_sb = const_pool.tile([IN_F, N], F32)
            nc.sync.dma_start(xt_sb[:], xt[:])
            wt_sb = const_pool.tile([IN_F, OUT_F], F32)
            nc.sync.dma_start(wt_sb[:], wt[:])

            at_r3 = at.rearrange("(c j p) n -> c p j n", p=P, j=jc)

            v_sb = v_pool.tile([P, MC, OUT_F], a_dt, tag="v_sb")
            with tc.tile_pool(name="psv", bufs=2, space="PSUM") as psv_pool:
                for mc in range(MC):
                    ps_v = psv_pool.tile([P, OUT_F], F32)
                    nc.tensor.matmul(
                        ps_v[:],
                        xt_sb[:, mc * P : (mc + 1) * P],
                        wt_sb[:],
                        start=True,
                        stop=True,
                    )
                    nc.vector.tensor_copy(v_sb[:, mc, :], ps_v[:])

            with tc.tile_pool(name="psy", bufs=1, space="PSUM") as psy_pool:
                for _rep in range(repeat):
                    out_sb = out_pool.tile([OUT_F, N], F32, tag="out_sb")
                    ps_y = psy_pool.tile([OUT_F, N], F32, tag="ps_y")
                    for c in range(MC // jc):
                        at_t = at_pool.tile([P, jc, N], a_dt)
                        eng = nc.scalar if (alt and c % 2) else nc.sync
                        eng.dma_start(at_t[:], at_r3[c])
                        for j in range(jc):
                            mc = c * jc + j
                            for b in range(NB):
                                nc.tensor.matmul(
                                    ps_y[:, b * BANK : (b + 1) * BANK],
                                    v_sb[:, mc, :],
                                    at_t[:, j, b * BANK : (b + 1) * BANK],
                                    start=(mc == 0),
                                    stop=(mc == MC - 1),
                                )
                                if mc == MC - 1:
                                    nc.vector.tensor_copy(
                                        out_sb[:, b * BANK : (b + 1) * BANK],
                                        ps_y[:, b * BANK : (b + 1) * BANK],
                                    )
                                    nc.sync.dma_start(
                                        ytp[:, b * BANK : (b + 1) * BANK],
                                        out_sb[:, b * BANK : (b + 1) * BANK],
                                    )

    nc.compile()
    return nc


def run_with_results(inputs, repeat=1, mode=None):
    """Run the kernel; returns (full_output [4096, 64] f32, BassKernelResults)."""
    mode = mode or MODE
    adjacency = np.asarray(inputs["adjacency"], dtype=np.float32)
    x = np.asarray(inputs["x"], dtype=np.float32)
    weight = np.asarray(inputs["weight"], dtype=np.float32)
    assert adjacency.shape == (R, N, N)
    assert x.shape == (N, IN_F)
    assert weight.shape == (R, OUT_F, IN_F)

    in_maps = make_in_maps(adjacency, x, weight, mode)

    key = (repeat, mode)
    if key not in _NC_CACHE:
        _NC_CACHE[key] = _build_nc(repeat, mode)
    nc = _NC_CACHE[key]

    res = run_bass_kernel_spmd(nc, in_maps, core_ids=list(range(R)))
    return assemble_output(res.results), res


def make_in_maps(adjacency, x, weight, mode=None):
    mode = mode or MODE
    import ml_dtypes

    # Host-side layout prep: contraction dim must land on SBUF partitions.
    if mode == "fp8r":
        # Row-sharded: core c owns output rows [512c, 512(c+1)) across all 8
        # relations. A is pre-swizzled into DMA stream order: contraction
        # chunk kc (= r*32 + mc) of core c's A.T slice lands at stream chunk
        # t = 2*kc if kc < 128 else 2*(kc-128)+1.
        jc = 8
        xt_np = np.ascontiguousarray(x.T).astype(ml_dtypes.bfloat16)
        wt_np = np.ascontiguousarray(
            weight.transpose(2, 0, 1).reshape(IN_F, R * OUT_F)
        ).astype(ml_dtypes.bfloat16)
        maps = []
        for c in range(R):
            asl = adjacency[:, c * BANK : (c + 1) * BANK, :]     # [R, 512, N]
            chunks = np.ascontiguousarray(asl.transpose(0, 2, 1)).reshape(
                R * MC, P, BANK
            )                                                    # [kc, p, n]
            stream = np.empty_like(chunks)
            stream[0::2] = chunks[: R * MC // 2]
            stream[1::2] = chunks[R * MC // 2 :]
            slabs = np.ascontiguousarray(
                stream.reshape(R * MC // jc, jc, P, BANK).transpose(0, 2, 1, 3)
            ).reshape(R * MC // jc, jc * P * BANK)
            maps.append(
                {
                    "at": slabs.astype(ml_dtypes.float8_e3m4),
                    "xt": xt_np,
                    "wt": wt_np,
                }
            )
        return maps

    at_np = np.ascontiguousarray(adjacency.transpose(0, 2, 1))  # [R, m, n]
    if mode.startswith("fp8"):
        at_np = at_np.astype(ml_dtypes.float8_e3m4)
        xt_np = np.ascontiguousarray(x.T).astype(ml_dtypes.bfloat16)
        wt_np = np.ascontiguousarray(weight.transpose(0, 2, 1)).astype(
            ml_dtypes.bfloat16
        )
    else:
        if mode == "bf16":
            at_np = at_np.astype(ml_dtypes.bfloat16)
        xt_np = np.ascontiguousarray(x.T)                       # [IN_F, N]
        wt_np = np.ascontiguousarray(weight.transpose(0, 2, 1))  # [R, IN_F, OUT_F]
    return [{"at": at_np[r], "xt": xt_np, "wt": wt_np[r]} for r in range(R)]


def assemble_output(results):
    if results[0]["ytp"].shape == (2 * OUT_F, BANK):
        # fp8r: core c returns the two column-group partials of its own
        # 512-row slice of y (transposed); sum the partials and concatenate.
        y = np.empty((N, OUT_F), dtype=np.float32)
        for c in range(R):
            p = results[c]["ytp"].astype(np.float32)
            y[c * BANK : (c + 1) * BANK] = (p[:OUT_F] + p[OUT_F:]).T
        return y
    yt = np.zeros((OUT_F, N), dtype=np.float32)
    for r in range(R):
        yt += results[r]["ytp"].astype(np.float32)
    return np.ascontiguousarray(yt.T)


def kernel(**inputs) -> np.ndarray:
    y, _ = run_with_results(inputs)
    return y


# revision 22
# speedup vs baseline: 2.6838x; 1.1294x over previous
"""Trainium2 Bass kernel for relational graph convolution:

    y = sum_r (A[r] @ x) @ W[r].T        A: [8, 4096, 4096] f32
                                         x: [4096, 64] f32, W: [8, 64, 64] f32

Strategy
--------
By associativity, y = sum_r A[r] @ v_r with v_r = x @ W[r].T, turning the
problem into one [4096, 4096] @ [4096, 64] matmul per relation. Relations are
sharded across the 8 NeuronCores (expert-style parallelism); each core returns
its partial y_r.T and the host sums and transposes.

The TensorE contracts over the partition dimension of both operands, so the
contraction index m (A's column index) must land on SBUF partitions. The host
therefore ships A[r].T (row-major) so device DMAs are plain contiguous slabs.

MODE="fp8r" (default): A is shipped as fp8 e3m4 (1 byte/elem, 16 MB/core --
quarter of the f32 DMA traffic; measured end-to-end rel error ~8.8e-3 on the
graded inputs, under the 2e-2 gate because the stationary operand v stays
bf16 -- bass allows mixed-dtype matmuls for all non-fp32 dtypes). Sharding is
by OUTPUT ROWS rather than relation: core c computes the disjoint 512-row
slice y[512c:512(c+1)] over ALL 8 relations (contraction K = R*N = 32768), so
the per-pass store is 128 KB bf16 instead of a 512 KB full-y partial, and the
host just sums two column-group partials and concatenates. The PE side uses
2x column tiling: the two PE column groups (tile_position (0,0) / (0,64)) each
accumulate half of the interleaved contraction-chunk stream concurrently,
halving PE streaming time to ~28 us so the kernel rides the ~430 GB/s/core
DMA roofline (~37 us for 16.1 MB). Measured 38.6 us/pass vs 142.9 us for the
f32r baseline.

Per core (fp8r mode):
  phase 1: v_r = x @ W_r.T for all 8 r via 32 bf16 matmuls (lhsT = x.T column
           chunks, rhs = [W_0.T | ... | W_7.T]), rounded to bf16 in SBUF.
  phase 2: 32 slab DMAs (512 KB each, host-preswizzled into DMA stream order,
           alternating the two HWDGE rings); per 128-row stream chunk one fp8
           matmul, issued as concurrent column-group pairs, accumulates
           y_slice.T into PSUM [128, 512] (group 0 = even stream chunks,
           group 1 = odd).
  phase 3: one DVE + one ACT [64, 512] PSUM -> SBUF copy, one 128 KB store.

MODE="fp8" is the relation-sharded fp8 variant (42.2 us), MODE="f32r" /
MODE="bf16" the earlier exact-ish variants (kept for fallback / reference).
"""

import numpy as np

import concourse.tile as tile
from concourse import bacc, mybir
from concourse.bass_utils import run_bass_kernel_spmd

R, N, IN_F, OUT_F = 8, 4096, 64, 64
P = 128            # partition dim / contraction chunk
MC = N // P        # 32 contraction chunks
BANK = 512         # fp32 elems per PSUM bank
NB = N // BANK     # 8 output column blocks
NB2 = NB // 2      # 4 column blocks per PE column group (fp8 mode)
HALF = N // 2

F32 = mybir.dt.float32
BF16 = mybir.dt.bfloat16
FP8 = mybir.dt.float8e3

MODE = "fp8r"      # "fp8r" (default), "fp8", "f32r", or "bf16"

_NC_CACHE = {}


def _build_nc_fp8(repeat=1, jc=4, at_bufs=8, probe=None):
    """fp8 e3m4 A + bf16 v, 2x column-tiled PE. jc = 128-row chunks of A per
    DMA slab (jc=4 -> 2 MB transfers). probe='dma' drops all phase-2 compute
    (pure A-stream bandwidth), probe='pe' DMAs one slab once and re-reads it
    (pure PE throughput); both are diagnostics, not graded paths."""
    nc = bacc.Bacc("TRN2", target_bir_lowering=False, debug=False, num_devices=R)

    at = nc.dram_tensor("at", [N, N], FP8, kind="ExternalInput").ap()
    xt = nc.dram_tensor("xt", [IN_F, N], BF16, kind="ExternalInput").ap()
    wt = nc.dram_tensor("wt", [IN_F, OUT_F], BF16, kind="ExternalInput").ap()
    # bf16 partial outputs: halves the store traffic; the host accumulates the
    # 8 per-relation partials in fp32 so the extra rounding is ~1e-3 rel_fro.
    ytp = nc.dram_tensor("ytp", [OUT_F, N], BF16, kind="ExternalOutput").ap()

    with tile.TileContext(nc) as tc:
        with (
            tc.tile_pool(name="const", bufs=1) as const_pool,
            tc.tile_pool(name="atp", bufs=at_bufs) as at_pool,
            tc.tile_pool(name="vp", bufs=2) as v_pool,
            tc.tile_pool(name="outp", bufs=2) as out_pool,
        ):
            xt_sb = const_pool.tile([IN_F, N], BF16)
            nc.sync.dma_start(xt_sb[:], xt[:])
            wt_sb = const_pool.tile([IN_F, OUT_F], BF16)
            nc.sync.dma_start(wt_sb[:], wt[:])

            at_r3 = at.rearrange("(c j p) n -> c p j n", p=P, j=jc)

            # phase 1: v[m, o] = sum_i x[m, i] W[o, i] in bf16.
            v_sb = v_pool.tile([P, MC, OUT_F], BF16, tag="v_sb")
            with tc.tile_pool(name="psv", bufs=2, space="PSUM") as psv_pool:
                for mc in range(MC):
                    ps_v = psv_pool.tile([P, OUT_F], F32)
                    nc.tensor.matmul(
                        ps_v[:],
                        xt_sb[:, mc * P : (mc + 1) * P],
                        wt_sb[:],
                        start=True,
                        stop=True,
                    )
                    nc.vector.tensor_copy(v_sb[:, mc, :], ps_v[:])

            # phase 2: y_r.T[o, n] += sum_m v[m, o] * A_r.T[m, n], with the
            # two PE column groups handling output columns [0, 2048) and
            # [2048, 4096) concurrently.
            at_fixed = None
            if probe == "pe":
                at_fixed = const_pool.tile([P, jc, N], FP8)
                nc.sync.dma_start(at_fixed[:], at_r3[0])
            with tc.tile_pool(name="psy", bufs=2, space="PSUM") as psy_pool:
                for _rep in range(repeat):
                    out_sb = out_pool.tile([P, HALF], BF16, tag="out_sb")
                    ps_y = psy_pool.tile([P, HALF], F32, tag="ps_y")
                    for c in range(MC // jc):
                        if probe == "pe":
                            at_t = at_fixed
                        else:
                            at_t = at_pool.tile([P, jc, N], FP8)
                            eng = nc.scalar if (c % 2) else nc.sync
                            eng.dma_start(at_t[:], at_r3[c])
                        if probe == "dma":
                            continue
                        for j in range(jc):
                            mc = c * jc + j
                            for b in range(NB2):
                                sl = slice(b * BANK, (b + 1) * BANK)
                                nc.tensor.matmul(
                                    ps_y[0:OUT_F, sl],
                                    v_sb[:, mc, :],
                                    at_t[:, j, sl],
                                    start=(mc == 0),
                                    stop=(mc == MC - 1),
                                )
                                nc.tensor.matmul(
                                    ps_y[OUT_F : 2 * OUT_F, sl],
                                    v_sb[:, mc, :],
                                    at_t[:, j, HALF + b * BANK : HALF + (b + 1) * BANK],
                                    start=(mc == 0),
                                    stop=(mc == MC - 1),
                                )
                    # phase 3: per-region copies chase the final matmuls; DVE
                    # takes col group 0, ACT col group 1, staggered two PSUM
                    # banks apart so the engines never contend on one bank.
                    if probe == "dma":
                        continue
                    # (stores ride the scalar HWDGE ring so the next rep's
                    # first A slab -- queued on the sync ring -- isn't stuck
                    # behind them)
                    for b in range(NB2):
                        sl = slice(b * BANK, (b + 1) * BANK)
                        nc.vector.tensor_copy(out_sb[0:OUT_F, sl], ps_y[0:OUT_F, sl])
                        nc.scalar.dma_start(ytp[:, sl], out_sb[0:OUT_F, sl])
                        b2 = (b + 2) % NB2
                        sl2 = slice(b2 * BANK, (b2 + 1) * BANK)
                        nc.scalar.activation(
                            out_sb[OUT_F : 2 * OUT_F, sl2],
                            ps_y[OUT_F : 2 * OUT_F, sl2],
                            mybir.ActivationFunctionType.Copy,
                        )
                        nc.scalar.dma_start(
                            ytp[:, HALF + b2 * BANK : HALF + (b2 + 1) * BANK],
                            out_sb[OUT_F : 2 * OUT_F, sl2],
                        )

    nc.compile()
    return nc


def _build_nc_fp8r(repeat=1, jc=8, at_bufs=6):
    """Row-sharded variant: each core computes a disjoint 512-row slice of y
    across ALL 8 relations (contraction K = R*N = 32768), instead of a full-y
    partial for one relation. Same 16 MB/core of A traffic, but the per-rep
    output drops from 512 KB to 128 KB (the two PE column-group partials
    [2, 64, 512], summed on the host) and the store tail is one region.

    The host ships A pre-swizzled in DMA order: stream chunk t interleaves the
    two contraction halves (t even -> chunk t/2, t odd -> chunk 128 + t/2) so
    the column-group pair (grp0, grp1) consumes consecutive stream chunks and
    compute chases the DMA stream at chunk granularity."""
    CH = 2 * MC * R // 2   # 256 stream chunks of 128 contraction rows
    PAIRS = CH // 2        # 128 concurrent matmul pairs
    CJ = CH // jc          # DMA slabs

    nc = bacc.Bacc("TRN2", target_bir_lowering=False, debug=False, num_devices=R)

    at = nc.dram_tensor("at", [CJ, jc * P * BANK], FP8, kind="ExternalInput").ap()
    xt = nc.dram_tensor("xt", [IN_F, N], BF16, kind="ExternalInput").ap()
    # wt[i, r*64+o] = W[r][o, i]: all 8 relations' W.T side by side
    wt = nc.dram_tensor("wt", [IN_F, R * OUT_F], BF16, kind="ExternalInput").ap()
    ytp = nc.dram_tensor("ytp", [2 * OUT_F, BANK], BF16, kind="ExternalOutput").ap()

    def chunk_info(t):
        grp, pair = t % 2, t // 2
        kc = pair if grp == 0 else PAIRS + pair
        r, mc = divmod(kc, MC)
        return grp, pair, r, mc

    with tile.TileContext(nc) as tc:
        with (
            tc.tile_pool(name="const", bufs=1) as const_pool,
            tc.tile_pool(name="atp", bufs=at_bufs) as at_pool,
            tc.tile_pool(name="vp", bufs=2) as v_pool,
            tc.tile_pool(name="outp", bufs=2) as out_pool,
        ):
            xt_sb = const_pool.tile([IN_F, N], BF16)
            nc.sync.dma_start(xt_sb[:], xt[:])
            wt_sb = const_pool.tile([IN_F, R * OUT_F], BF16)
            nc.sync.dma_start(wt_sb[:], wt[:])

            at_r = at.rearrange("d (p j n) -> d p j n", p=P, j=jc)

            # phase 1: v for all 8 relations; one [128, 512] matmul per row
            # chunk computes x_chunk @ [W_0.T | ... | W_7.T].
            v_sb = v_pool.tile([P, MC, R * OUT_F], BF16, tag="v_sb")
            with tc.tile_pool(name="psv", bufs=2, space="PSUM") as psv_pool:
                for mc in range(MC):
                    ps_v = psv_pool.tile([P, R * OUT_F], F32)
                    nc.tensor.matmul(
                        ps_v[:],
                        xt_sb[:, mc * P : (mc + 1) * P],
                        wt_sb[:],
                        start=True,
                        stop=True,
                    )
                    nc.vector.tensor_copy(v_sb[:, mc, :], ps_v[:])

            # phase 2: y_slice.T[o, n] = sum_{r,m} v_r[m, o] * A_r.T[m, n],
            # col group 0 accumulating stream chunks t even, group 1 t odd.
            with tc.tile_pool(name="psy", bufs=2, space="PSUM") as psy_pool:
                for _rep in range(repeat):
                    out_sb = out_pool.tile([P, BANK], BF16, tag="out_sb")
                    ps_y = psy_pool.tile([P, BANK], F32, tag="ps_y")
                    for d in range(CJ):
                        at_t = at_pool.tile([P, jc, BANK], FP8)
                        eng = nc.scalar if (d % 2) else nc.sync
                        eng.dma_start(at_t[:], at_r[d])
                        for j in range(jc):
                            t = d * jc + j
                            grp, pair, r, mc = chunk_info(t)
                            nc.tensor.matmul(
                                ps_y[grp * OUT_F : (grp + 1) * OUT_F, :],
                                v_sb[:, mc, r * OUT_F : (r + 1) * OUT_F],
                                at_t[:, j, :],
                                start=(pair == 0),
                                stop=(pair == PAIRS - 1),
                            )
                    nc.vector.tensor_copy(out_sb[0:OUT_F, :], ps_y[0:OUT_F, :])
                    nc.scalar.activation(
                        out_sb[OUT_F : 2 * OUT_F, :],
                        ps_y[OUT_F : 2 * OUT_F, :],
                        mybir.ActivationFunctionType.Copy,
                    )
                    nc.scalar.dma_start(ytp[:], out_sb[:])

    nc.compile()
    return nc


def _build_nc(repeat=1, mode=None, jc=None, alt=True, at_bufs=None):
    """repeat>1 re-runs phase 2 (the steady-state A-streaming loop) that many
    times inside one NEFF -- used only by the benchmark harness to amortize
    per-execute dispatch overhead; the graded kernel uses repeat=1."""
    mode = mode or MODE
    if mode.startswith("fp8r"):
        # tuning variants: fp8r16 = 1 MB slabs, fp8rb = deeper slab lookahead
        cfg = {
            "fp8r": (8, 6),
            "fp8r16": (16, 4),
            "fp8rb": (8, 12),
        }[mode]
        return _build_nc_fp8r(repeat, jc=jc or cfg[0], at_bufs=at_bufs or cfg[1])
    if mode.startswith("fp8"):
        probe = {"fp8": None, "fp8_dma": "dma", "fp8_pe": "pe"}[mode]
        return _build_nc_fp8(repeat, jc=jc or 4, at_bufs=at_bufs or 8, probe=probe)

    a_dt = mybir.dt.float32r if mode == "f32r" else mybir.dt.bfloat16
    if jc is None:
        jc = 1 if mode == "f32r" else 2
    if at_bufs is None:
        at_bufs = {1: 4, 2: 3, 4: 2}[jc] if mode == "f32r" else 4

    nc = bacc.Bacc("TRN2", target_bir_lowering=False, debug=False, num_devices=R)

    at = nc.dram_tensor("at", [N, N], a_dt, kind="ExternalInput").ap()
    xt = nc.dram_tensor("xt", [IN_F, N], F32, kind="ExternalInput").ap()
    wt = nc.dram_tensor("wt", [IN_F, OUT_F], F32, kind="ExternalInput").ap()
    ytp = nc.dram_tensor("ytp", [OUT_F, N], F32, kind="ExternalOutput").ap()

    with tile.TileContext(nc) as tc:
        with (
            tc.tile_pool(name="const", bufs=1) as const_pool,
            tc.tile_pool(name="atp", bufs=at_bufs) as at_pool,
            tc.tile_pool(name="vp", bufs=2) as v_pool,
            tc.tile_pool(name="outp", bufs=2) as out_pool,
        ):
            xt_sb = const_pool.tile([IN_F, N], F32)
            nc.sync.dma_start(xt_sb[:], xt[:])
            wt_sb = const_pool.tile([IN_F, OUT_F], F32)
            nc.sync.dma_start(wt_sb[:], wt[:])

            at_r3 = at.rearrange("(c j p) n -> c p j n", p=P, j=jc)

            v_sb = v_pool.tile([P, MC, OUT_F], a_dt, tag="v_sb")
            with tc.tile_pool(name="psv", bufs=2, space="PSUM") as psv_pool:
                for mc in range(MC):
                    ps_v = psv_pool.tile([P, OUT_F], F32)
                    nc.tensor.matmul(
                        ps_v[:],
                        xt_sb[:, mc * P : (mc + 1) * P],
                        wt_sb[:],
                        start=True,
                        stop=True,
                    )
                    nc.vector.tensor_copy(v_sb[:, mc, :], ps_v[:])

            with tc.tile_pool(name="psy", bufs=1, space="PSUM") as psy_pool:
                for _rep in range(repeat):
                    out_sb = out_pool.tile([OUT_F, N], F32, tag="out_sb")
                    ps_y = psy_pool.tile([OUT_F, N], F32, tag="ps_y")
                    for c in range(MC // jc):
                        at_t = at_pool.tile([P, jc, N], a_dt)
                        eng = nc.scalar if (alt and c % 2) else nc.sync
                        eng.dma_start(at_t[:], at_r3[c])
                        for j in range(jc):
                            mc = c * jc + j
                            for b in range(NB):
                                nc.tensor.matmul(
                                    ps_y[:, b * BANK : (b + 1) * BANK],
                                    v_sb[:, mc, :],
                                    at_t[:, j, b * BANK : (b + 1) * BANK],
                                    start=(mc == 0),
                                    stop=(mc == MC - 1),
                                )
                                if mc == MC - 1:
                                    nc.vector.tensor_copy(
                                        out_sb[:, b * BANK : (b + 1) * BANK],
                                        ps_y[:, b * BANK : (b + 1) * BANK],
                                    )
                                    nc.sync.dma_start(
                                        ytp[:, b * BANK : (b + 1) * BANK],
                                        out_sb[:, b * BANK : (b + 1) * BANK],
                                    )

    nc.compile()
    return nc


def run_with_results(inputs, repeat=1, mode=None):
    """Run the kernel; returns (full_output [4096, 64] f32, BassKernelResults)."""
    mode = mode or MODE
    adjacency = np.asarray(inputs["adjacency"], dtype=np.float32)
    x = np.asarray(inputs["x"], dtype=np.float32)
    weight = np.asarray(inputs["weight"], dtype=np.float32)
    assert adjacency.shape == (R, N, N)
    assert x.shape == (N, IN_F)
    assert weight.shape == (R, OUT_F, IN_F)

    in_maps = make_in_maps(adjacency, x, weight, mode)

    key = (repeat, mode)
    if key not in _NC_CACHE:
        _NC_CACHE[key] = _build_nc(repeat, mode)
    nc = _NC_CACHE[key]

    res = run_bass_kernel_spmd(nc, in_maps, core_ids=list(range(R)))
    return assemble_output(res.results), res


def make_in_maps(adjacency, x, weight, mode=None):
    mode = mode or MODE
    import ml_dtypes

    # Host-side layout prep: contraction dim must land on SBUF partitions.
    if mode.startswith("fp8r"):
        # Row-sharded: core c owns output rows [512c, 512(c+1)) across all 8
        # relations. A is pre-swizzled into DMA stream order: contraction
        # chunk kc (= r*32 + mc) of core c's A.T slice lands at stream chunk
        # t = 2*kc if kc < 128 else 2*(kc-128)+1.
        jc = 16 if mode == "fp8r16" else 8
        xt_np = np.ascontiguousarray(x.T).astype(ml_dtypes.bfloat16)
        wt_np = np.ascontiguousarray(
            weight.transpose(2, 0, 1).reshape(IN_F, R * OUT_F)
        ).astype(ml_dtypes.bfloat16)
        maps = []
        for c in range(R):
            asl = adjacency[:, c * BANK : (c + 1) * BANK, :]     # [R, 512, N]
            chunks = np.ascontiguousarray(asl.transpose(0, 2, 1)).reshape(
                R * MC, P, BANK
            )                                                    # [kc, p, n]
            stream = np.empty_like(chunks)
            stream[0::2] = chunks[: R * MC // 2]
            stream[1::2] = chunks[R * MC // 2 :]
            slabs = np.ascontiguousarray(
                stream.reshape(R * MC // jc, jc, P, BANK).transpose(0, 2, 1, 3)
            ).reshape(R * MC // jc, jc * P * BANK)
            maps.append(
                {
                    "at": slabs.astype(ml_dtypes.float8_e3m4),
                    "xt": xt_np,
                    "wt": wt_np,
                }
            )
        return maps

    at_np = np.ascontiguousarray(adjacency.transpose(0, 2, 1))  # [R, m, n]
    if mode.startswith("fp8"):
        at_np = at_np.astype(ml_dtypes.float8_e3m4)
        xt_np = np.ascontiguousarray(x.T).astype(ml_dtypes.bfloat16)
        wt_np = np.ascontiguousarray(weight.transpose(0, 2, 1)).astype(
            ml_dtypes.bfloat16
        )
    else:
        if mode == "bf16":
            at_np = at_np.astype(ml_dtypes.bfloat16)
        xt_np = np.ascontiguousarray(x.T)                       # [IN_F, N]
        wt_np = np.ascontiguousarray(weight.transpose(0, 2, 1))  # [R, IN_F, OUT_F]
    return [{"at": at_np[r], "xt": xt_np, "wt": wt_np[r]} for r in range(R)]


def assemble_output(results):
    if results[0]["ytp"].shape == (2 * OUT_F, BANK):
        # fp8r: core c returns the two column-group partials of its own
        # 512-row slice of y (transposed); sum the partials and concatenate.
        y = np.empty((N, OUT_F), dtype=np.float32)
        for c in range(R):
            p = results[c]["ytp"].astype(np.float32)
            y[c * BANK : (c + 1) * BANK] = (p[:OUT_F] + p[OUT_F:]).T
        return y
    yt = np.zeros((OUT_F, N), dtype=np.float32)
    for r in range(R):
        yt += results[r]["ytp"].astype(np.float32)
    return np.ascontiguousarray(yt.T)


def kernel(**inputs) -> np.ndarray:
    y, _ = run_with_results(inputs)
    return y
